# revision 1
# baseline (speedup 1.0000x reference)
"""Trainium2 Bass kernel for GQA attention (B=4, S=1024, DIM=2048, 32 Q heads /
8 KV heads, head_dim 64, rotary + causal mask, QKV + output projections).

Sharding: 8 cores = batch (4) x head-half (2). Each core computes one batch's
attention for 16 Q heads / 4 KV heads plus the partial output projection over
its 1024 y-features; the host sums the two partials per batch.

Layout: feature-major (q^T/k^T/y^T), scores computed transposed (P^T[k, q]) so
softmax sums come from a ones-column in the AV stationary. Matmuls run in
float32r (fp32 storage, 11-bit mantissa) except AV which runs bf16 (P and v).
Causal structure is data-driven from the mask input (fully-masked 128-blocks
are skipped, mixed blocks get a multiplicative mask tile).
"""

import hashlib
import sys

import numpy as np

for _p in ("/root/.axon_site/_ro/trn_rl_repo", "/opt/trn_rl_repo"):
    if _p not in sys.path:
        sys.path.append(_p)

import ml_dtypes
import concourse.bacc as bacc
import concourse.mybir as mybir
from concourse.tile import TileContext
from concourse.bass_utils import run_bass_kernel_spmd

F32 = mybir.dt.float32
F32R = mybir.dt.float32r
BF16 = mybir.dt.bfloat16
AF = mybir.ActivationFunctionType
OP = mybir.AluOpType

B, S, DIM = 4, 1024, 2048
NH, NKV, HD = 32, 8, 64
NQL, NKVL = 16, 4
N_CORES = 8
KT = S // 128
QPAIRS = 8
ND = DIM // 128
SCALE = 1.0 / 8.0


def _pairing(t):
    return (t, t + 4) if t < 4 else (t + 4, t + 8)


def _analyze_mask(M):
    """Block-classify the [S, S] bool mask (M[q, k]).

    Returns:
      runs[ki]  : list of (qs, qe, [(qt, mask_idx)]) maximal valid runs over q
      span[ki]  : (lo, hi) overall valid q range or None
      mixed     : list of unique mixed-block tiles in P^T layout [k, q]
    """
    runs, span = {}, {}
    mixed, midx = [], {}
    for ki in range(KT):
        rr, cur = [], None
        lo = hi = None
        for qt in range(KT):
            blk = M[128 * qt:128 * qt + 128, 128 * ki:128 * ki + 128]
            if (~blk).all():
                if cur is not None:
                    rr.append(tuple(cur))
                    cur = None
                continue
            mix = []
            if not blk.all():
                key = blk.tobytes()
                if key not in midx:
                    mixed.append(np.ascontiguousarray(blk.T).astype(np.float32))
                    midx[key] = len(mixed) - 1
                mix = [(qt, midx[key])]
            if cur is None:
                cur = [128 * qt, 128 * qt + 128, mix]
            else:
                cur[1] = 128 * qt + 128
                cur[2] += mix
            lo = 128 * qt if lo is None else lo
            hi = 128 * qt + 128
        if cur is not None:
            rr.append(tuple(cur))
        runs[ki] = rr
        span[ki] = (lo, hi) if lo is not None else None
    return runs, span, mixed


def _bank_subruns(runs_ki):
    """Split runs at 512 boundaries -> [(qs, qe, qc)], each inside one bank."""
    out = []
    for (qs, qe, _mix) in runs_ki:
        for qc in range(2):
            a, b = max(qs, 512 * qc), min(qe, 512 * qc + 512)
            if a < b:
                out.append((a, b, qc))
    return out


def _build_program(runs, span, n_mixed):
    nc = bacc.Bacc("TRN2", target_bir_lowering=False, debug=False,
                   num_devices=N_CORES)

    xT = nc.dram_tensor("xT", [DIM, S], F32R, kind="ExternalInput")
    wqk = nc.dram_tensor("wqk", [10, 128, ND, 128], F32R, kind="ExternalInput")
    wv = nc.dram_tensor("wv", [ND, 128, NKVL * HD], F32R, kind="ExternalInput")
    wo_t = nc.dram_tensor("wo_t", [4, 128, 8, 512], F32R, kind="ExternalInput")
    cos_d = nc.dram_tensor("cos_d", [128, S], F32, kind="ExternalInput")
    sin_d = nc.dram_tensor("sin_d", [128, S], F32, kind="ExternalInput")
    nmx = max(n_mixed, 1)
    msk_d = nc.dram_tensor("msk_d", [nmx, 128, 128], BF16, kind="ExternalInput")
    out_d = nc.dram_tensor("out", [KT, 128, DIM], F32, kind="ExternalOutput")

    with TileContext(nc) as tc:
      with tc.tile_pool(name="res", bufs=1) as res:
        qk_t = [res.tile([128, S], F32R, name=f"qk{t}", tag=f"qk{t}")
                for t in range(10)]
        v_t = [res.tile([128, NKVL * 65], BF16, name=f"v{k}", tag=f"v{k}")
               for k in range(KT)]
        y_t = [res.tile([128, S], F32R, name=f"y{t}", tag=f"y{t}")
               for t in range(QPAIRS)]
        cos_sb = res.tile([128, S], F32, name="cos_sb")
        sin_sb = res.tile([128, S], F32, name="sin_sb")
        nc.sync.dma_start(cos_sb[:], cos_d[:])
        nc.sync.dma_start(sin_sb[:], sin_d[:])
        msk_sb = [res.tile([128, 128], BF16, name=f"msk{i}", tag=f"msk{i}")
                  for i in range(n_mixed)]
        for i in range(n_mixed):
            nc.sync.dma_start(msk_sb[i][:], msk_d[i])
        ones4 = res.tile([128, NKVL], F32, name="ones4")
        nc.gpsimd.memset(ones4[:], 1.0)
        woeh = [res.tile([128, 4, 512], F32R, name=f"woeh{i}", tag="woeh",
                         bufs=2) for i in range(8)]
        osb_t = [res.tile([128, 512], F32, name=f"osb{i}", tag="osb", bufs=2)
                 for i in range(32)]

        def qkproj(st, pp, half, f, xd, dest):
            """Project feature tile f for one token half + rope into dest."""
            tsl = slice(512 * half, 512 * half + 512)
            wf = st.tile([128, ND, 128], F32R, name=f"wf{half}_{f}", tag="wf",
                         bufs=2)
            nc.sync.dma_start(wf[:], wqk[f])
            ps = pp.tile([128, 512], F32, name=f"psqk{half}_{f}", tag="psproj",
                         bufs=2)
            for d in range(ND):
                nc.tensor.matmul(ps[:], wf[:, d, :], xd[d][:],
                                 start=(d == 0), stop=(d == ND - 1))
            c0 = st.tile([128, 512], F32, name=f"c0_{half}_{f}", tag="c0", bufs=2)
            nc.scalar.copy(c0[:], ps[:])
            sw = st.tile([128, 512], F32, name=f"sw_{half}_{f}", tag="sw", bufs=2)
            for blk in range(4):
                sb = (blk ^ 1) * 32
                nc.sync.dma_start(sw[32 * blk:32 * blk + 32, :],
                                  c0[sb:sb + 32, :])
            t1 = st.tile([128, 512], F32, name=f"t1_{half}_{f}", tag="t1", bufs=1)
            t2 = st.tile([128, 512], F32, name=f"t2_{half}_{f}", tag="t2", bufs=1)
            nc.vector.tensor_mul(t1[:], c0[:], cos_sb[:, tsl])
            nc.vector.tensor_mul(t2[:], sw[:], sin_sb[:, tsl])
            nc.vector.tensor_add(dest[:, tsl], t1[:], t2[:])

        # ------------- projections + attention (one pool scope) -------
        with (
            tc.tile_pool(name="p2", bufs=1) as st,
            tc.tile_pool(name="pp2", bufs=1, space="PSUM") as pp,
        ):
            with nc.named_scope("proj"):
                for half in range(2):
                    xd = [st.tile([128, 512], F32R, name=f"x2_{half}_{d}",
                                  tag="xd2", bufs=16) for d in range(ND)]
                    for d in range(ND):
                        nc.sync.dma_start(
                            xd[d][:],
                            xT[128 * d:128 * d + 128,
                               512 * half:512 * half + 512])
                    wvt = st.tile([128, ND, NKVL * HD], F32R,
                                  name=f"wvt{half}", tag="wvt", bufs=1)
                    nc.sync.dma_start(wvt[:], wv[:].rearrange("d p c -> p d c"))
                    for tq in range(4):
                        ki = 4 * half + tq
                        psv = pp.tile([128, NKVL * HD], F32, name=f"psv{ki}",
                                      tag="psproj", bufs=2)
                        for d in range(ND):
                            nc.tensor.matmul(
                                psv[:], xd[d][:, 128 * tq:128 * tq + 128],
                                wvt[:, d, :], start=(d == 0),
                                stop=(d == ND - 1))
                        vv = v_t[ki][:].rearrange("p (u c) -> p u c",
                                                  u=NKVL, c=65)
                        nc.scalar.copy(
                            vv[:, :, 0:64],
                            psv[:].rearrange("p (u c) -> p u c", u=NKVL, c=HD))
                        nc.scalar.copy(
                            vv[:, :, 64:65],
                            ones4[:].rearrange("p (u o) -> p u o",
                                               u=NKVL, o=1))
                    for f in [8, 9] + list(range(QPAIRS)):
                        qkproj(st, pp, half, f, xd, qk_t[f])

            with nc.named_scope("attn"):
                for p in range(QPAIRS):
                    tk = 0 if p < 4 else 1
                    heads = _pairing(p)
                    ptiles = {}
                    poff = {}
                    for ki in range(KT):
                        if span[ki] is None:
                            continue
                        lo, hi = span[ki]
                        w = hi - lo
                        kwin = slice(128 * ki, 128 * ki + 128)
                        merge = w <= 512
                        if merge:
                            # both heads share one PSUM tile (side s at
                            # psum col 512s) and one exp into a packed P tile
                            psS = pp.tile([128, 1024], F32,
                                          name=f"psS{p}{ki}", tag="psS", bufs=2)
                            pt = st.tile([128, 2 * w], BF16, name=f"P{p}_{ki}",
                                         tag=f"Pm_{ki}", bufs=1)
                            for s in range(2):
                                ptiles[(s, ki)] = pt
                                poff[(s, ki)] = s * w - lo
                                bank_first = True
                                for (qs, qe, qc) in _bank_subruns(runs[ki]):
                                    nc.tensor.matmul(
                                        psS[:, qs - lo + 512 * s:
                                            qe - lo + 512 * s],
                                        qk_t[8 + tk][64 * s:64 * s + 64, kwin],
                                        qk_t[p][64 * s:64 * s + 64, qs:qe],
                                        start=bank_first, stop=True,
                                        skip_group_check=True)
                                    bank_first = False
                            psv2 = psS[:].rearrange("p (b c) -> p b c",
                                                    b=2, c=512)[:, :, 0:w]
                            ptv = pt[:].rearrange("p (b c) -> p b c",
                                                  b=2, c=w)
                            nc.scalar.activation(ptv, psv2, AF.Exp,
                                                 scale=SCALE)
                            for (qs, qe, mix) in runs[ki]:
                                for (qt, mi) in mix:
                                    for s in range(2):
                                        o = s * w + 128 * qt - lo
                                        nc.vector.tensor_mul(
                                            pt[:, o:o + 128],
                                            pt[:, o:o + 128], msk_sb[mi][:])
                        else:
                            for s in range(2):
                                psS = pp.tile([128, 1024], F32,
                                              name=f"psS{p}{ki}{s}",
                                              tag="psS", bufs=2)
                                pt = st.tile([128, w], BF16,
                                             name=f"P{p}_{s}_{ki}",
                                             tag=f"P{s}_{ki}", bufs=1)
                                ptiles[(s, ki)] = pt
                                poff[(s, ki)] = -lo
                                bank_first = {}
                                for (qs, qe, qc) in _bank_subruns(runs[ki]):
                                    st_flag = bank_first.setdefault(qc, True)
                                    bank_first[qc] = False
                                    nc.tensor.matmul(
                                        psS[:, qs:qe],
                                        qk_t[8 + tk][64 * s:64 * s + 64, kwin],
                                        qk_t[p][64 * s:64 * s + 64, qs:qe],
                                        start=st_flag, stop=True,
                                        skip_group_check=True)
                                nc.scalar.activation(pt[:], psS[:, lo:hi],
                                                     AF.Exp, scale=SCALE)
                                for (qs, qe, mix) in runs[ki]:
                                    for (qt, mi) in mix:
                                        o = 128 * qt - lo
                                        nc.vector.tensor_mul(
                                            pt[:, o:o + 128],
                                            pt[:, o:o + 128], msk_sb[mi][:])
                    for s in range(2):
                        u = heads[s] // 4
                        for qc in range(2):
                            subs = []
                            for ki in range(KT):
                                if span[ki] is None:
                                    continue
                                for (qs, qe, qq) in _bank_subruns(runs[ki]):
                                    if qq == qc:
                                        subs.append((ki, qs, qe))
                            if not subs:
                                continue
                            psyf = pp.tile([128, 512], F32,
                                           name=f"psy{p}{s}{qc}", tag="psy",
                                           bufs=2)
                            psy = psyf[0:65, :]
                            for n, (ki, qs, qe) in enumerate(subs):
                                off = poff[(s, ki)]
                                nc.tensor.matmul(
                                    psy[:, qs - 512 * qc:qe - 512 * qc],
                                    v_t[ki][:, 65 * u:65 * u + 65],
                                    ptiles[(s, ki)][:, qs + off:qe + off],
                                    start=(n == 0), stop=(n == len(subs) - 1),
                                    skip_group_check=True)
                            s_sb = st.tile([1, 512], F32, name=f"s{p}{s}{qc}",
                                           tag="srf", bufs=2)
                            nc.vector.tensor_copy(s_sb[:], psy[64:65, :])
                            rf = st.tile([1, 512], F32, name=f"rf{p}{s}{qc}",
                                         tag="srf", bufs=2)
                            nc.vector.reciprocal_approx_fast(rf[:], s_sb[:])
                            rb = st.tile([64, 512], F32, name=f"rb{p}{s}{qc}",
                                         tag="rb", bufs=2)
                            nc.gpsimd.partition_broadcast(rb[:], rf[:])
                            nc.vector.tensor_mul(
                                y_t[p][64 * s:64 * s + 64,
                                       512 * qc:512 * qc + 512],
                                psy[0:64, :], rb[:])

        # ---------------- output projection ----------------
        with (
            nc.named_scope("wo"),
            tc.tile_pool(name="pp3", bufs=1, space="PSUM") as pp,
        ):
            for i in range(8):
                ec, fh = i // 2, i % 2
                nc.sync.dma_start(woeh[i][:], wo_t[ec, :, 4 * fh:4 * fh + 4, :])
            for ec in range(4):
                psos = {}
                for fh in range(2):
                    for tt in range(KT):
                        if fh == 0:
                            psos[tt] = pp.tile([128, 512], F32,
                                               name=f"pso{ec}{tt}", tag="pso",
                                               bufs=8)
                        pso = psos[tt]
                        for f in range(4 * fh, 4 * fh + 4):
                            nc.tensor.matmul(
                                pso[:], y_t[f][:, 128 * tt:128 * tt + 128],
                                woeh[2 * ec + fh][:, f % 4, :],
                                start=(f == 0), stop=(f == 7))
                        if fh == 1:
                            osb = osb_t[8 * ec + tt % 8]
                            nc.scalar.copy(osb[:], pso[:])
                            nc.sync.dma_start(
                                out_d[tt, :, 512 * ec:512 * ec + 512], osb[:])

    nc.compile()
    return nc


_CACHE = {}


def _get_program(mask):
    M = np.asarray(mask).reshape(S, S).astype(bool)
    key = hashlib.md5(M.tobytes()).hexdigest()
    if key not in _CACHE:
        runs, span, mixed = _analyze_mask(M)
        nc = _build_program(runs, span, len(mixed))
        _CACHE[key] = (nc, mixed)
    return _CACHE[key]


def _round_fp32r(a):
    """Round fp32 -> fp32r encoding (11-bit mantissa, low 12 bits zero)."""
    b = np.ascontiguousarray(a, dtype=np.float32).view(np.uint32)
    lsb = (b >> np.uint32(12)) & np.uint32(1)
    r = (b + np.uint32(0x7FF) + lsb) & np.uint32(0xFFFFF000)
    return r.view(np.float32)


def _host_inputs(x, freqs_cis, wqkv, wo, mixed):
    """Build the 8 per-core input maps."""
    x = np.asarray(x, dtype=np.float32)
    fc = np.asarray(freqs_cis, dtype=np.float32)
    wqkv = np.asarray(wqkv, dtype=np.float32)
    wo = np.asarray(wo, dtype=np.float32)

    cosv = fc[:, :, 0].T
    sinv = fc[:, :, 1].T
    cos_t = np.ascontiguousarray(np.tile(cosv, (4, 1)))
    sin_t = np.tile(sinv, (4, 1))
    sgn = np.ones((128, 1), np.float32)
    sgn[np.arange(128) % 64 < 32] = -1.0
    sin_t = np.ascontiguousarray(sin_t * sgn)

    nmx = max(len(mixed), 1)
    msk_arr = np.zeros((nmx, 128, 128), ml_dtypes.bfloat16)
    for i, m in enumerate(mixed):
        msk_arr[i] = m.astype(ml_dtypes.bfloat16)

    j = np.arange(HD)
    refdim = 2 * (j % 32) + (j // 32)

    in_maps = []
    for b in range(B):
        xTb = np.ascontiguousarray(x[b].T)
        for h in range(2):
            rows = np.empty(1280, np.int64)
            for t in range(8):
                a, bb = _pairing(t)
                for sde, ql in enumerate((a, bb)):
                    g = h * NQL + ql
                    rows[128 * t + 64 * sde + j] = g * HD + refdim
            for tkk in range(2):
                for sde in range(2):
                    u = tkk * 2 + sde
                    g = h * NKVL + u
                    rows[1024 + 128 * tkk + 64 * sde + j] = \
                        NH * HD + g * HD + refdim
            W4 = wqkv[rows]
            wqk_a = np.ascontiguousarray(
                W4.reshape(10, 128, ND, 128).transpose(0, 3, 2, 1))
            vrows = (NH + NKV) * HD + (h * NKVL * HD) + np.arange(NKVL * HD)
            Wv = wqkv[vrows]
            wv_a = np.ascontiguousarray(
                Wv.reshape(NKVL * HD, ND, 128).transpose(1, 2, 0))
            worow = np.empty(1024, np.int64)
            dd = np.arange(HD)
            for t in range(8):
                a, bb = _pairing(t)
                for sde, ql in enumerate((a, bb)):
                    worow[128 * t + 64 * sde + dd] = (h * NQL + ql) * HD + dd
            woT = np.ascontiguousarray(wo[:, worow].T)
            wo_a = np.ascontiguousarray(
                woT.reshape(8, 128, 4, 512).transpose(2, 1, 0, 3))
            in_maps.append({
                "xT": _round_fp32r(xTb),
                "wqk": _round_fp32r(wqk_a),
                "wv": _round_fp32r(wv_a),
                "wo_t": _round_fp32r(wo_a),
                "cos_d": cos_t,
                "sin_d": sin_t,
                "msk_d": msk_arr,
            })
    return in_maps


def _run(x, freqs_cis, mask, wqkv, wo, trace=False):
    nc, mixed = _get_program(mask)
    in_maps = _host_inputs(x, freqs_cis, wqkv, wo, mixed)
    res = run_bass_kernel_spmd(nc, in_maps, list(range(N_CORES)), trace=trace)
    outs = [res.results[i]["out"].reshape(S, DIM) for i in range(N_CORES)]
    full = np.stack([outs[2 * b] + outs[2 * b + 1] for b in range(B)])
    return full.astype(np.float32), res


def kernel(x, freqs_cis, mask, wqkv, wo):
    full, _ = _run(x, freqs_cis, mask, wqkv, wo, trace=False)
    return full



# revision 2
# speedup vs baseline: 1.6771x; 1.6771x over previous
"""Trainium2 Bass kernel for GQA attention (B=4, S=1024, DIM=2048, 32 Q heads /
8 KV heads, head_dim 64, rotary + causal mask, QKV + output projections).

Sharding: 8 cores = batch (4) x head-half (2). Each core computes one batch's
attention for 16 Q heads / 4 KV heads plus the partial output projection over
its 1024 y-features; the host sums the two partials per batch.

v2: all-bf16 dataflow (weights/x/q/k/P/v/y/wo/out in bf16, fp32 PSUM accum).
PE emission is interleaved to stay dense: proj f-groups alternate with
attention over the first token bank (qc0), then wo for tokens 0-511 overlaps
attention over the second bank (qc1), then wo for tokens 512-1023. Scores are
computed transposed (P^T[k, q]) with softmax sums from a ones-column in the AV
stationary. Causal structure is data-driven from the mask input.
"""

import hashlib
import sys

import numpy as np

for _p in ("/root/.axon_site/_ro/trn_rl_repo", "/opt/trn_rl_repo"):
    if _p not in sys.path:
        sys.path.append(_p)

import ml_dtypes
import concourse.bacc as bacc
import concourse.mybir as mybir
from concourse.tile import TileContext
from concourse.bass_utils import run_bass_kernel_spmd

F32 = mybir.dt.float32
BF16 = mybir.dt.bfloat16
AF = mybir.ActivationFunctionType

B, S, DIM = 4, 1024, 2048
NH, NKV, HD = 32, 8, 64
NQL, NKVL = 16, 4
N_CORES = 8
KT = S // 128
QPAIRS = 8
ND = DIM // 128
SCALE = 1.0 / 8.0


def _pairing(t):
    return (t, t + 4) if t < 4 else (t + 4, t + 8)


def _analyze_mask(M):
    """Block-classify the [S, S] bool mask (M[q, k]).

    Returns:
      runs[ki]  : list of (qs, qe, [(qt, mask_idx)]) maximal valid runs over q
      mixed     : list of unique mixed-block tiles in P^T layout [k, q]
    """
    runs = {}
    mixed, midx = [], {}
    for ki in range(KT):
        rr, cur = [], None
        for qt in range(KT):
            blk = M[128 * qt:128 * qt + 128, 128 * ki:128 * ki + 128]
            if (~blk).all():
                if cur is not None:
                    rr.append(tuple(cur))
                    cur = None
                continue
            mix = []
            if not blk.all():
                key = blk.tobytes()
                if key not in midx:
                    mixed.append(np.ascontiguousarray(blk.T).astype(np.float32))
                    midx[key] = len(mixed) - 1
                mix = [(qt, midx[key])]
            if cur is None:
                cur = [128 * qt, 128 * qt + 128, mix]
            else:
                cur[1] = 128 * qt + 128
                cur[2] += mix
            hi = 128 * qt + 128
        if cur is not None:
            rr.append(tuple(cur))
        runs[ki] = rr
    return runs, mixed


def _clip_runs(runs_ki, qc):
    """Clip runs to bank qc; return (a0, a1, rr) or None."""
    lo, hi = 512 * qc, 512 * qc + 512
    rr = []
    for (qs, qe, mix) in runs_ki:
        a, b = max(qs, lo), min(qe, hi)
        if a < b:
            rr.append((a, b, [(qt, mi) for (qt, mi) in mix
                              if a <= 128 * qt < b]))
    if not rr:
        return None
    return rr[0][0], rr[-1][1], rr


def _build_program(runs, n_mixed):
    nc = bacc.Bacc("TRN2", target_bir_lowering=False, debug=False,
                   num_devices=N_CORES)

    xt4 = nc.dram_tensor("xt4", [2, ND, 128, 512], BF16, kind="ExternalInput")
    wqk = nc.dram_tensor("wqk", [10, 128, ND, 128], BF16, kind="ExternalInput")
    wv = nc.dram_tensor("wv", [ND, 128, NKVL * HD], BF16, kind="ExternalInput")
    wo_t = nc.dram_tensor("wo_t", [4, 128, 8, 512], BF16, kind="ExternalInput")
    cos_d = nc.dram_tensor("cos_d", [128, S], BF16, kind="ExternalInput")
    sin_d = nc.dram_tensor("sin_d", [128, S], BF16, kind="ExternalInput")
    nmx = max(n_mixed, 1)
    msk_d = nc.dram_tensor("msk_d", [nmx, 128, 256], BF16, kind="ExternalInput")
    out_d = nc.dram_tensor("out", [KT, 128, DIM], BF16, kind="ExternalOutput")

    with TileContext(nc) as tc:
      with (
          tc.tile_pool(name="res", bufs=1) as res,
          tc.tile_pool(name="pp", bufs=1, space="PSUM") as pp,
      ):
        # ---------------- persistent tiles + prioritized DMA ----------
        qk_t = [res.tile([128, S], BF16, name=f"qk{t}", tag=f"qk{t}")
                for t in range(10)]
        v_t = [res.tile([128, NKVL * 65], BF16, name=f"v{k}", tag=f"v{k}")
               for k in range(KT)]
        y_t = [res.tile([128, S], BF16, name=f"y{t}", tag=f"y{t}")
               for t in range(QPAIRS)]
        wf = [res.tile([128, ND, 128], BF16, name=f"wf{f}", tag="wf", bufs=3)
              for f in range(10)]
        xd = [[res.tile([128, 512], BF16, name=f"x{h}_{d}", tag=f"x{h}_{d}")
               for d in range(ND)] for h in range(2)]
        cos_sb = res.tile([128, S], BF16, name="cos_sb")
        sin_sb = res.tile([128, S], BF16, name="sin_sb")
        wvt = res.tile([128, ND, NKVL * HD], BF16, name="wvt")
        woeh = [res.tile([128, 4, 512], BF16, name=f"woeh{i}", tag=f"woeh{i}")
                for i in range(8)]
        msk_sb = [res.tile([128, 256], BF16, name=f"msk{i}", tag=f"msk{i}")
                  for i in range(n_mixed)]
        ones4 = res.tile([128, NKVL], BF16, name="ones4")
        osb_t = [res.tile([128, 512], BF16, name=f"osb{i}", tag="osb", bufs=4)
                 for i in range(32)]

        # DMA priority order: first proj weights + x, then tables, then rest.
        # wf has bufs=3, so only wf[8], wf[9], wf[0] load now; later wf DMAs
        # are emitted just before their f-group (buffer cycling gates them).
        fseq = [8, 9] + list(range(8))
        for f in fseq[:3]:
            nc.sync.dma_start(wf[f][:], wqk[f])
        for h in range(2):
            for d in range(ND):
                nc.sync.dma_start(xd[h][d][:], xt4[h, d])
        nc.sync.dma_start(cos_sb[:], cos_d[:])
        nc.sync.dma_start(sin_sb[:], sin_d[:])
        nc.sync.dma_start(wvt[:], wv[:].rearrange("d p c -> p d c"))
        for i in range(n_mixed):
            nc.sync.dma_start(msk_sb[i][:], msk_d[i])
        for i in range(8):
            ec, fh = i // 2, i % 2
            nc.sync.dma_start(woeh[i][:], wo_t[ec, :, 4 * fh:4 * fh + 4, :])
        nc.gpsimd.memset(ones4[:], 1.0)

        # ---------------- emission helpers ----------------
        def proj(fi, h):
            """Project feature tile fseq[fi] for token half h + rope."""
            f = fseq[fi]
            tsl = slice(512 * h, 512 * h + 512)
            if fi + 3 < len(fseq) and h == 1:
                nc.sync.dma_start(wf[fseq[fi + 3]][:], wqk[fseq[fi + 3]])
            ps = pp.tile([128, 512], F32, name=f"ps{h}_{f}", tag="big", bufs=2)
            for d in range(ND):
                nc.tensor.matmul(ps[:], wf[f][:, d, :], xd[h][d][:],
                                 start=(d == 0), stop=(d == ND - 1))
            c0 = st_tile([128, 512], BF16, f"c0_{h}_{f}", "c0", 2)
            nc.scalar.copy(c0[:], ps[:])
            sw = st_tile([128, 512], BF16, f"sw_{h}_{f}", "sw", 2)
            for blk in range(4):
                sb = (blk ^ 1) * 32
                nc.sync.dma_start(sw[32 * blk:32 * blk + 32, :],
                                  c0[sb:sb + 32, :])
            t1 = st_tile([128, 512], BF16, f"t1_{h}_{f}", "t1", 2)
            t2 = st_tile([128, 512], BF16, f"t2_{h}_{f}", "t2", 2)
            nc.vector.tensor_mul(t1[:], c0[:], cos_sb[:, tsl])
            nc.vector.tensor_mul(t2[:], sw[:], sin_sb[:, tsl])
            nc.vector.tensor_add(qk_t[f][:, tsl], t1[:], t2[:])

        def st_tile(shape, dt, name, tag, bufs):
            return res.tile(shape, dt, name=name, tag=tag, bufs=bufs)

        def vproj(tq):
            """Value projection for token tile tq (0..7)."""
            h, t = tq // 4, tq % 4
            psv = pp.tile([128, NKVL * HD], F32, name=f"psv{tq}", tag="big",
                          bufs=2)
            for d in range(ND):
                nc.tensor.matmul(psv[:], xd[h][d][:, 128 * t:128 * t + 128],
                                 wvt[:, d, :], start=(d == 0),
                                 stop=(d == ND - 1))
            vv = v_t[tq][:].rearrange("p (u c) -> p u c", u=NKVL, c=65)
            nc.scalar.copy(vv[:, :, 0:64],
                           psv[:].rearrange("p (u c) -> p u c", u=NKVL, c=HD))
            nc.scalar.copy(vv[:, :, 64:65],
                           ones4[:].rearrange("p (u o) -> p u o", u=NKVL, o=1))

        pts = {}

        def qk_phase(p, qc):
            """QK matmuls + exp + diag-mask for pair p, token bank qc."""
            tk = 0 if p < 4 else 1
            for ki in range(KT):
                cl = _clip_runs(runs[ki], qc)
                if cl is None:
                    continue
                a0, a1, rr = cl
                w = a1 - a0
                kwin = slice(128 * ki, 128 * ki + 128)
                psS = pp.tile([128, 1024], F32, name=f"psS{p}{qc}{ki}",
                              tag="psS", bufs=2)
                for s in range(2):
                    first = True
                    for (qs, qe, _mix) in rr:
                        nc.tensor.matmul(
                            psS[:, 512 * s + qs - 512 * qc:
                                512 * s + qe - 512 * qc],
                            qk_t[8 + tk][64 * s:64 * s + 64, kwin],
                            qk_t[p][64 * s:64 * s + 64, qs:qe],
                            start=first, stop=True, skip_group_check=True)
                        first = False
                pt = st_tile([128, 2, w], BF16, f"P{p}_{qc}_{ki}",
                             f"pt{qc}_{ki}", 2)
                pts[(p, qc, ki)] = (pt, a0)
                psv2 = psS[:].rearrange("p (b c) -> p b c", b=2, c=512)
                nc.scalar.activation(
                    pt[:], psv2[:, :, a0 - 512 * qc:a1 - 512 * qc],
                    AF.Exp, scale=SCALE)
                for (qs, qe, mix) in rr:
                    for (qt, mi) in mix:
                        o = 128 * qt - a0
                        m2 = msk_sb[mi][:].rearrange("p (a b) -> p a b", a=2)
                        nc.vector.tensor_mul(pt[:, :, o:o + 128],
                                             pt[:, :, o:o + 128], m2)

        def av_phase(p, qc):
            """AV matmuls + softmax normalization for pair p, bank qc."""
            heads = _pairing(p)
            for s in range(2):
                u = heads[s] // 4
                subs = []
                for ki in range(KT):
                    cl = _clip_runs(runs[ki], qc)
                    if cl is None:
                        continue
                    a0, a1, rr = cl
                    for (qs, qe, _mix) in rr:
                        subs.append((ki, qs, qe, a0))
                psyf = pp.tile([128, 512], F32, name=f"psy{p}{s}{qc}",
                               tag="psy", bufs=2)
                psy = psyf[0:65, :]
                for n, (ki, qs, qe, a0) in enumerate(subs):
                    pt = pts[(p, qc, ki)][0]
                    nc.tensor.matmul(
                        psy[:, qs - 512 * qc:qe - 512 * qc],
                        v_t[ki][:, 65 * u:65 * u + 65],
                        pt[:, s, qs - a0:qe - a0],
                        start=(n == 0), stop=(n == len(subs) - 1),
                        skip_group_check=True)
                s_sb = st_tile([1, 512], F32, f"s{p}{s}{qc}", "srf", 2)
                nc.vector.tensor_copy(s_sb[:], psy[64:65, :])
                rf = st_tile([1, 512], F32, f"rf{p}{s}{qc}", "srf", 2)
                nc.vector.reciprocal_approx_fast(rf[:], s_sb[:])
                rb = st_tile([64, 512], F32, f"rb{p}{s}{qc}", "rb", 2)
                nc.gpsimd.partition_broadcast(rb[:], rf[:])
                nc.vector.tensor_mul(
                    y_t[p][64 * s:64 * s + 64, 512 * qc:512 * qc + 512],
                    psy[0:64, :], rb[:])

        def wo_group(ec, tt):
            """Output projection for (out-col chunk ec, token tile tt)."""
            pso = pp.tile([128, 512], F32, name=f"pso{ec}{tt}", tag="big",
                          bufs=2)
            for f in range(8):
                nc.tensor.matmul(pso[:], y_t[f][:, 128 * tt:128 * tt + 128],
                                 woeh[2 * ec + f // 4][:, f % 4, :],
                                 start=(f == 0), stop=(f == 7))
            osb = osb_t[4 * tt + ec]
            nc.scalar.copy(osb[:], pso[:])
            nc.sync.dma_start(out_d[tt, :, 512 * ec:512 * ec + 512], osb[:])

        # ---------------- phase 1: proj + attention bank 0 -------------
        with nc.named_scope("p1"):
            for fi in range(3):          # k tiles (f=8,9) + q pair 0
                proj(fi, 0)
                proj(fi, 1)
            for tq in range(8):
                vproj(tq)
            qk_phase(0, 0)
            for p in range(QPAIRS):
                if p + 1 < QPAIRS:
                    proj(p + 3, 0)
                    proj(p + 3, 1)
                    qk_phase(p + 1, 0)
                av_phase(p, 0)

        # ---------------- phase 2: wo half 0 + attention bank 1 --------
        wo_q = [(ec, tt) for tt in range(4) for ec in range(4)]
        with nc.named_scope("p2"):
            qk_phase(0, 1)
            gi = 0
            for p in range(QPAIRS):
                if p + 1 < QPAIRS:
                    qk_phase(p + 1, 1)
                if p >= 1:
                    wo_group(*wo_q[gi]); gi += 1
                    wo_group(*wo_q[gi]); gi += 1
                av_phase(p, 1)
            while gi < len(wo_q):
                wo_group(*wo_q[gi]); gi += 1

        # ---------------- phase 3: wo half 1 ---------------------------
        with nc.named_scope("p3"):
            for tt in range(4, 8):
                for ec in range(4):
                    wo_group(ec, tt)

    nc.compile()
    return nc


_CACHE = {}


def _get_program(mask):
    M = np.asarray(mask).reshape(S, S).astype(bool)
    key = hashlib.md5(M.tobytes()).hexdigest()
    if key not in _CACHE:
        runs, mixed = _analyze_mask(M)
        nc = _build_program(runs, len(mixed))
        _CACHE[key] = (nc, mixed)
    return _CACHE[key]


def _host_inputs(x, freqs_cis, wqkv, wo, mixed):
    """Build the 8 per-core input maps (all bf16)."""
    bf = ml_dtypes.bfloat16
    x = np.asarray(x, dtype=np.float32)
    fc = np.asarray(freqs_cis, dtype=np.float32)
    wqkv = np.asarray(wqkv, dtype=np.float32)
    wo = np.asarray(wo, dtype=np.float32)

    cosv = fc[:, :, 0].T
    sinv = fc[:, :, 1].T
    cos_t = np.ascontiguousarray(np.tile(cosv, (4, 1))).astype(bf)
    sgn = np.ones((128, 1), np.float32)
    sgn[np.arange(128) % 64 < 32] = -1.0
    sin_t = np.ascontiguousarray(np.tile(sinv, (4, 1)) * sgn).astype(bf)

    nmx = max(len(mixed), 1)
    msk_arr = np.zeros((nmx, 128, 256), bf)
    for i, m in enumerate(mixed):
        msk_arr[i, :, 0:128] = m.astype(bf)
        msk_arr[i, :, 128:256] = m.astype(bf)

    j = np.arange(HD)
    refdim = 2 * (j % 32) + (j // 32)

    in_maps = []
    for b in range(B):
        xt4 = np.ascontiguousarray(
            x[b].T.reshape(ND, 128, 2, 512).transpose(2, 0, 1, 3)).astype(bf)
        for h in range(2):
            rows = np.empty(1280, np.int64)
            for t in range(8):
                a, bb = _pairing(t)
                for sde, ql in enumerate((a, bb)):
                    g = h * NQL + ql
                    rows[128 * t + 64 * sde + j] = g * HD + refdim
            for tkk in range(2):
                for sde in range(2):
                    u = tkk * 2 + sde
                    g = h * NKVL + u
                    rows[1024 + 128 * tkk + 64 * sde + j] = \
                        NH * HD + g * HD + refdim
            W4 = wqkv[rows]
            wqk_a = np.ascontiguousarray(
                W4.reshape(10, 128, ND, 128).transpose(0, 3, 2, 1)).astype(bf)
            vrows = (NH + NKV) * HD + (h * NKVL * HD) + np.arange(NKVL * HD)
            Wv = wqkv[vrows]
            wv_a = np.ascontiguousarray(
                Wv.reshape(NKVL * HD, ND, 128).transpose(1, 2, 0)).astype(bf)
            worow = np.empty(1024, np.int64)
            dd = np.arange(HD)
            for t in range(8):
                a, bb = _pairing(t)
                for sde, ql in enumerate((a, bb)):
                    worow[128 * t + 64 * sde + dd] = (h * NQL + ql) * HD + dd
            woT = np.ascontiguousarray(wo[:, worow].T)
            wo_a = np.ascontiguousarray(
                woT.reshape(8, 128, 4, 512).transpose(2, 1, 0, 3)).astype(bf)
            in_maps.append({
                "xt4": xt4,
                "wqk": wqk_a,
                "wv": wv_a,
                "wo_t": wo_a,
                "cos_d": cos_t,
                "sin_d": sin_t,
                "msk_d": msk_arr,
            })
    return in_maps


def _run(x, freqs_cis, mask, wqkv, wo, trace=False):
    nc, mixed = _get_program(mask)
    in_maps = _host_inputs(x, freqs_cis, wqkv, wo, mixed)
    res = run_bass_kernel_spmd(nc, in_maps, list(range(N_CORES)), trace=trace)
    outs = [res.results[i]["out"].astype(np.float32).reshape(S, DIM)
            for i in range(N_CORES)]
    full = np.stack([outs[2 * b] + outs[2 * b + 1] for b in range(B)])
    return full.astype(np.float32), res


def kernel(x, freqs_cis, mask, wqkv, wo):
    full, _ = _run(x, freqs_cis, mask, wqkv, wo, trace=False)
    return full


# revision 9
# speedup vs baseline: 1.7217x; 1.0266x over previous
"""Trainium2 Bass kernel for GQA attention (B=4, S=1024, DIM=2048, 32 Q heads /
8 KV heads, head_dim 64, rotary + causal mask, QKV + output projections).

Sharding: 8 cores = batch (4) x head-half (2). Each core computes one batch's
attention for 16 Q heads / 4 KV heads plus the partial output projection over
its 1024 y-features; the host sums the two partials per batch.

v2: all-bf16 dataflow (weights/x/q/k/P/v/y/wo/out in bf16, fp32 PSUM accum).
PE emission is interleaved to stay dense: proj f-groups alternate with
attention over the first token bank (qc0), then wo for tokens 0-511 overlaps
attention over the second bank (qc1), then wo for tokens 512-1023. Scores are
computed transposed (P^T[k, q]) with softmax sums from a ones-column in the AV
stationary. Causal structure is data-driven from the mask input.
"""

import hashlib
import sys

import numpy as np

for _p in ("/root/.axon_site/_ro/trn_rl_repo", "/opt/trn_rl_repo"):
    if _p not in sys.path:
        sys.path.append(_p)

import ml_dtypes
import concourse.bacc as bacc
import concourse.mybir as mybir
from concourse.tile import TileContext
from concourse.bass_utils import run_bass_kernel_spmd

F32 = mybir.dt.float32
BF16 = mybir.dt.bfloat16
AF = mybir.ActivationFunctionType

B, S, DIM = 4, 1024, 2048
NH, NKV, HD = 32, 8, 64
NQL, NKVL = 16, 4
N_CORES = 8
KT = S // 128
QPAIRS = 8
ND = DIM // 128
SCALE = 1.0 / 8.0


def _pairing(t):
    return (t, t + 4) if t < 4 else (t + 4, t + 8)


def _analyze_mask(M):
    """Block-classify the [S, S] bool mask (M[q, k]).

    Returns:
      runs[ki]  : list of (qs, qe, [(qt, mask_idx)]) maximal valid runs over q
      mixed     : list of unique mixed-block tiles in P^T layout [k, q]
    """
    runs = {}
    mixed, midx = [], {}
    for ki in range(KT):
        rr, cur = [], None
        for qt in range(KT):
            blk = M[128 * qt:128 * qt + 128, 128 * ki:128 * ki + 128]
            if (~blk).all():
                if cur is not None:
                    rr.append(tuple(cur))
                    cur = None
                continue
            mix = []
            if not blk.all():
                key = blk.tobytes()
                if key not in midx:
                    mixed.append(np.ascontiguousarray(blk.T).astype(np.float32))
                    midx[key] = len(mixed) - 1
                mix = [(qt, midx[key])]
            if cur is None:
                cur = [128 * qt, 128 * qt + 128, mix]
            else:
                cur[1] = 128 * qt + 128
                cur[2] += mix
            hi = 128 * qt + 128
        if cur is not None:
            rr.append(tuple(cur))
        runs[ki] = rr
    return runs, mixed


def _clip_runs(runs_ki, qc):
    """Clip runs to bank qc; return (a0, a1, rr) or None."""
    lo, hi = 512 * qc, 512 * qc + 512
    rr = []
    for (qs, qe, mix) in runs_ki:
        a, b = max(qs, lo), min(qe, hi)
        if a < b:
            rr.append((a, b, [(qt, mi) for (qt, mi) in mix
                              if a <= 128 * qt < b]))
    if not rr:
        return None
    return rr[0][0], rr[-1][1], rr


def _build_program(runs, n_mixed):
    nc = bacc.Bacc("TRN2", target_bir_lowering=False, debug=False,
                   num_devices=N_CORES)

    xt4 = nc.dram_tensor("xt4", [2, ND, 128, 512], BF16, kind="ExternalInput")
    wqk = nc.dram_tensor("wqk", [10, 128, ND, 128], BF16, kind="ExternalInput")
    wv = nc.dram_tensor("wv", [ND, 128, NKVL * HD], BF16, kind="ExternalInput")
    wo_t = nc.dram_tensor("wo_t", [4, 128, 8, 512], BF16, kind="ExternalInput")
    cos_d = nc.dram_tensor("cos_d", [128, S], BF16, kind="ExternalInput")
    sin_d = nc.dram_tensor("sin_d", [128, S], BF16, kind="ExternalInput")
    nmx = max(n_mixed, 1)
    msk_d = nc.dram_tensor("msk_d", [nmx, 128, 256], BF16, kind="ExternalInput")
    out_d = nc.dram_tensor("out", [KT, 128, DIM], BF16, kind="ExternalOutput")

    with TileContext(nc) as tc:
      with (
          tc.tile_pool(name="res", bufs=1) as res,
          tc.tile_pool(name="pp", bufs=1, space="PSUM") as pp,
      ):
        # ---------------- persistent tiles + prioritized DMA ----------
        qk_t = [res.tile([128, S], BF16, name=f"qk{t}", tag=f"qk{t}")
                for t in range(10)]
        v_t = [res.tile([128, NKVL * 65], BF16, name=f"v{k}", tag=f"v{k}")
               for k in range(KT)]
        y_t = [res.tile([128, S], BF16, name=f"y{t}", tag=f"y{t}")
               for t in range(QPAIRS)]
        wf = [res.tile([128, ND, 128], BF16, name=f"wf{f}", tag="wf", bufs=4)
              for f in range(10)]
        xd = [[res.tile([128, 512], BF16, name=f"x{h}_{d}", tag=f"x{h}_{d}")
               for d in range(ND)] for h in range(2)]
        cos_sb = res.tile([128, S], BF16, name="cos_sb")
        sin_sb = res.tile([128, S], BF16, name="sin_sb")
        wvt = res.tile([128, ND, NKVL * HD], BF16, name="wvt")
        woeh = [res.tile([128, 4, 512], BF16, name=f"woeh{i}", tag=f"woeh{i}")
                for i in range(8)]
        msk_sb = [res.tile([128, 256], BF16, name=f"msk{i}", tag=f"msk{i}")
                  for i in range(n_mixed)]
        ones4 = res.tile([128, NKVL], BF16, name="ones4")
        osb_t = [res.tile([128, 512], BF16, name=f"osb{i}", tag="osb", bufs=4)
                 for i in range(32)]

        # DMA priority order: first-needed first. woeh loads are deferred to
        # mid-phase-1; rope shuffle DMAs go via the Scalar DGE queue so they
        # never sit behind bulk loads on the Sync queue.
        fseq = [8, 9] + list(range(8))
        nc.sync.dma_start(wf[8][:], wqk[8])
        for d in range(ND):
            nc.sync.dma_start(xd[0][d][:], xt4[0, d])
        nc.sync.dma_start(wf[9][:], wqk[9])
        nc.sync.dma_start(cos_sb[:], cos_d[:])
        nc.sync.dma_start(sin_sb[:], sin_d[:])
        nc.sync.dma_start(wf[0][:], wqk[0])
        nc.sync.dma_start(wf[1][:], wqk[1])
        for d in range(ND):
            nc.sync.dma_start(xd[1][d][:], xt4[1, d])
        nc.sync.dma_start(wvt[:], wv[:].rearrange("d p c -> p d c"))
        for i in range(n_mixed):
            nc.sync.dma_start(msk_sb[i][:], msk_d[i])
        nc.gpsimd.memset(ones4[:], 1.0)

        # ---------------- emission helpers ----------------
        wf_next = [4]

        def proj(fi, h):
            """Project feature tile fseq[fi] for token half h + rope."""
            f = fseq[fi]
            tsl = slice(512 * h, 512 * h + 512)
            if h == 1 and wf_next[0] < len(fseq):
                nc.sync.dma_start(wf[fseq[wf_next[0]]][:],
                                  wqk[fseq[wf_next[0]]])
                wf_next[0] += 1
            ps = pp.tile([128, 512], F32, name=f"ps{h}_{f}", tag="big", bufs=2)
            for d in range(ND):
                nc.tensor.matmul(ps[:], wf[f][:, d, :], xd[h][d][:],
                                 start=(d == 0), stop=(d == ND - 1))
            c0 = st_tile([128, 512], BF16, f"c0_{h}_{f}", "c0", 2)
            nc.scalar.copy(c0[:], ps[:])
            sw = st_tile([128, 512], BF16, f"sw_{h}_{f}", "sw", 2)
            for blk in range(4):
                sb = (blk ^ 1) * 32
                nc.scalar.dma_start(sw[32 * blk:32 * blk + 32, :],
                                    c0[sb:sb + 32, :])
            t1 = st_tile([128, 512], BF16, f"t1_{h}_{f}", "t1", 2)
            t2 = st_tile([128, 512], BF16, f"t2_{h}_{f}", "t2", 2)
            nc.vector.tensor_mul(t1[:], c0[:], cos_sb[:, tsl])
            nc.vector.tensor_mul(t2[:], sw[:], sin_sb[:, tsl])
            nc.vector.tensor_add(qk_t[f][:, tsl], t1[:], t2[:])

        def st_tile(shape, dt, name, tag, bufs):
            return res.tile(shape, dt, name=name, tag=tag, bufs=bufs)

        def vproj(tq):
            """Value projection for token tile tq (0..7)."""
            h, t = tq // 4, tq % 4
            psv = pp.tile([128, NKVL * HD], F32, name=f"psv{tq}", tag="big",
                          bufs=2)
            for d in range(ND):
                nc.tensor.matmul(psv[:], xd[h][d][:, 128 * t:128 * t + 128],
                                 wvt[:, d, :], start=(d == 0),
                                 stop=(d == ND - 1))
            vv = v_t[tq][:].rearrange("p (u c) -> p u c", u=NKVL, c=65)
            nc.scalar.copy(vv[:, :, 0:64],
                           psv[:].rearrange("p (u c) -> p u c", u=NKVL, c=HD))
            nc.scalar.copy(vv[:, :, 64:65],
                           ones4[:].rearrange("p (u o) -> p u o", u=NKVL, o=1))

        pts = {}

        def qk_phase(p, qc, kis=None):
            """QK matmuls + exp + diag-mask for pair p, token bank qc."""
            tk = 0 if p < 4 else 1
            for ki in (range(KT) if kis is None else kis):
                cl = _clip_runs(runs[ki], qc)
                if cl is None:
                    continue
                a0, a1, rr = cl
                w = a1 - a0
                kwin = slice(128 * ki, 128 * ki + 128)
                psS = pp.tile([128, 1024], F32, name=f"psS{p}{qc}{ki}",
                              tag="psS", bufs=2)
                for s in range(2):
                    first = True
                    for (qs, qe, _mix) in rr:
                        nc.tensor.matmul(
                            psS[:, 512 * s + qs - 512 * qc:
                                512 * s + qe - 512 * qc],
                            qk_t[8 + tk][64 * s:64 * s + 64, kwin],
                            qk_t[p][64 * s:64 * s + 64, qs:qe],
                            start=first, stop=True, skip_group_check=True)
                        first = False
                pt = st_tile([128, 2, w], BF16, f"P{p}_{qc}_{ki}",
                             f"pt{qc}_{ki}", 2)
                pts[(p, qc, ki)] = (pt, a0)
                psv2 = psS[:].rearrange("p (b c) -> p b c", b=2, c=512)
                nc.scalar.activation(
                    pt[:], psv2[:, :, a0 - 512 * qc:a1 - 512 * qc],
                    AF.Exp, scale=SCALE)
                for (qs, qe, mix) in rr:
                    for (qt, mi) in mix:
                        o = 128 * qt - a0
                        m2 = msk_sb[mi][:].rearrange("p (a b) -> p a b", a=2)
                        nc.vector.tensor_mul(pt[:, :, o:o + 128],
                                             pt[:, :, o:o + 128], m2)

        def av_phase(p, qc):
            """AV matmuls + softmax normalization for pair p, bank qc."""
            heads = _pairing(p)
            for s in range(2):
                u = heads[s] // 4
                subs = []
                for ki in range(KT):
                    cl = _clip_runs(runs[ki], qc)
                    if cl is None:
                        continue
                    a0, a1, rr = cl
                    for (qs, qe, _mix) in rr:
                        subs.append((ki, qs, qe, a0))
                psyf = pp.tile([128, 512], F32, name=f"psy{p}{s}{qc}",
                               tag="psy", bufs=2)
                psy = psyf[0:65, :]
                for n, (ki, qs, qe, a0) in enumerate(subs):
                    pt = pts[(p, qc, ki)][0]
                    nc.tensor.matmul(
                        psy[:, qs - 512 * qc:qe - 512 * qc],
                        v_t[ki][:, 65 * u:65 * u + 65],
                        pt[:, s, qs - a0:qe - a0],
                        start=(n == 0), stop=(n == len(subs) - 1),
                        skip_group_check=True)
                s_sb = st_tile([1, 512], F32, f"s{p}{s}{qc}", "srf", 2)
                nc.vector.tensor_copy(s_sb[:], psy[64:65, :])
                rf = st_tile([1, 512], F32, f"rf{p}{s}{qc}", "srf", 2)
                nc.vector.reciprocal_approx_fast(rf[:], s_sb[:])
                rb = st_tile([64, 512], F32, f"rb{p}{s}{qc}", "rb", 2)
                nc.gpsimd.partition_broadcast(rb[:], rf[:])
                nc.vector.tensor_mul(
                    y_t[p][64 * s:64 * s + 64, 512 * qc:512 * qc + 512],
                    psy[0:64, :], rb[:])

        def wo_group(ec, tt):
            """Output projection for (out-col chunk ec, token tile tt)."""
            pso = pp.tile([128, 512], F32, name=f"pso{ec}{tt}", tag="big",
                          bufs=2)
            for f in range(8):
                nc.tensor.matmul(pso[:], y_t[f][:, 128 * tt:128 * tt + 128],
                                 woeh[2 * ec + f // 4][:, f % 4, :],
                                 start=(f == 0), stop=(f == 7))
            osb = osb_t[4 * tt + ec]
            nc.scalar.copy(osb[:], pso[:])
            nc.sync.dma_start(out_d[tt, :, 512 * ec:512 * ec + 512], osb[:])

        # ---------------- phase 1: proj + attention bank 0 -------------
        with nc.named_scope("p1"):
            proj(0, 0)                   # k tiles (f=8,9) + q pair 0
            proj(1, 0)
            proj(0, 1)
            proj(1, 1)
            proj(2, 0)
            proj(2, 1)
            for tq in range(8):
                vproj(tq)
            qk_phase(0, 0)
            for p in range(QPAIRS):
                if p + 1 < QPAIRS:
                    proj(p + 3, 0)
                    proj(p + 3, 1)
                    qk_phase(p + 1, 0)
                if p == 3:
                    for i in range(8):
                        ec, fh = i // 2, i % 2
                        nc.sync.dma_start(woeh[i][:],
                                          wo_t[ec, :, 4 * fh:4 * fh + 4, :])
                av_phase(p, 0)

        # ---------------- phase 2: wo half 0 + attention bank 1 --------
        wo_q = [(ec, tt) for tt in range(4) for ec in range(4)]
        with nc.named_scope("p2"):
            gi = 0
            qk_phase(0, 1, [0, 1])
            wo_group(*wo_q[gi]); gi += 1
            qk_phase(0, 1, [2, 3])
            wo_group(*wo_q[gi]); gi += 1
            qk_phase(0, 1, [4, 5, 6, 7])
            for p in range(QPAIRS - 1):
                qk_phase(p + 1, 1, [0, 1])
                wo_group(*wo_q[gi]); gi += 1
                qk_phase(p + 1, 1, [2, 3])
                av_phase(p, 1)
                qk_phase(p + 1, 1, [4, 5, 6, 7])
                wo_group(*wo_q[gi]); gi += 1
            av_phase(QPAIRS - 1, 1)

        # ---------------- phase 3: wo half 1 ---------------------------
        with nc.named_scope("p3"):
            for tt in range(4, 8):
                for ec in range(4):
                    wo_group(ec, tt)

    nc.compile()
    return nc


_CACHE = {}


def _get_program(mask):
    M = np.asarray(mask).reshape(S, S).astype(bool)
    key = hashlib.md5(M.tobytes()).hexdigest()
    if key not in _CACHE:
        runs, mixed = _analyze_mask(M)
        nc = _build_program(runs, len(mixed))
        _CACHE[key] = (nc, mixed)
    return _CACHE[key]


def _host_inputs(x, freqs_cis, wqkv, wo, mixed):
    """Build the 8 per-core input maps (all bf16)."""
    bf = ml_dtypes.bfloat16
    x = np.asarray(x, dtype=np.float32)
    fc = np.asarray(freqs_cis, dtype=np.float32)
    wqkv = np.asarray(wqkv, dtype=np.float32)
    wo = np.asarray(wo, dtype=np.float32)

    cosv = fc[:, :, 0].T
    sinv = fc[:, :, 1].T
    cos_t = np.ascontiguousarray(np.tile(cosv, (4, 1))).astype(bf)
    sgn = np.ones((128, 1), np.float32)
    sgn[np.arange(128) % 64 < 32] = -1.0
    sin_t = np.ascontiguousarray(np.tile(sinv, (4, 1)) * sgn).astype(bf)

    nmx = max(len(mixed), 1)
    msk_arr = np.zeros((nmx, 128, 256), bf)
    for i, m in enumerate(mixed):
        msk_arr[i, :, 0:128] = m.astype(bf)
        msk_arr[i, :, 128:256] = m.astype(bf)

    j = np.arange(HD)
    refdim = 2 * (j % 32) + (j // 32)

    in_maps = []
    for b in range(B):
        xt4 = np.ascontiguousarray(
            x[b].T.reshape(ND, 128, 2, 512).transpose(2, 0, 1, 3)).astype(bf)
        for h in range(2):
            rows = np.empty(1280, np.int64)
            for t in range(8):
                a, bb = _pairing(t)
                for sde, ql in enumerate((a, bb)):
                    g = h * NQL + ql
                    rows[128 * t + 64 * sde + j] = g * HD + refdim
            for tkk in range(2):
                for sde in range(2):
                    u = tkk * 2 + sde
                    g = h * NKVL + u
                    rows[1024 + 128 * tkk + 64 * sde + j] = \
                        NH * HD + g * HD + refdim
            W4 = wqkv[rows]
            wqk_a = np.ascontiguousarray(
                W4.reshape(10, 128, ND, 128).transpose(0, 3, 2, 1)).astype(bf)
            vrows = (NH + NKV) * HD + (h * NKVL * HD) + np.arange(NKVL * HD)
            Wv = wqkv[vrows]
            wv_a = np.ascontiguousarray(
                Wv.reshape(NKVL * HD, ND, 128).transpose(1, 2, 0)).astype(bf)
            worow = np.empty(1024, np.int64)
            dd = np.arange(HD)
            for t in range(8):
                a, bb = _pairing(t)
                for sde, ql in enumerate((a, bb)):
                    worow[128 * t + 64 * sde + dd] = (h * NQL + ql) * HD + dd
            woT = np.ascontiguousarray(wo[:, worow].T)
            wo_a = np.ascontiguousarray(
                woT.reshape(8, 128, 4, 512).transpose(2, 1, 0, 3)).astype(bf)
            in_maps.append({
                "xt4": xt4,
                "wqk": wqk_a,
                "wv": wv_a,
                "wo_t": wo_a,
                "cos_d": cos_t,
                "sin_d": sin_t,
                "msk_d": msk_arr,
            })
    return in_maps


def _run(x, freqs_cis, mask, wqkv, wo, trace=False):
    nc, mixed = _get_program(mask)
    in_maps = _host_inputs(x, freqs_cis, wqkv, wo, mixed)
    res = run_bass_kernel_spmd(nc, in_maps, list(range(N_CORES)), trace=trace)
    outs = [res.results[i]["out"].astype(np.float32).reshape(S, DIM)
            for i in range(N_CORES)]
    full = np.stack([outs[2 * b] + outs[2 * b + 1] for b in range(B)])
    return full.astype(np.float32), res


def kernel(x, freqs_cis, mask, wqkv, wo):
    full, _ = _run(x, freqs_cis, mask, wqkv, wo, trace=False)
    return full
